# revision 2
# baseline (speedup 1.0000x reference)
# MoE (8 experts, top-2) on 8 TRN2 NeuronCores — expert-parallel.
#
# Host (numpy): router matmul + softmax + top-2 (exactly mirrors the jax
# reference arithmetic in fp32), then dispatch: gather each expert's tokens
# into a padded [D, C] column block (bf16, pre-transposed for the device
# matmul layout).
# Device (per core, expert e): hT = gelu_tanh(W1[e]^T @ xT + b1), then
# y = (hT^T @ W2[e]) * gate — all matmuls bf16 with fp32 PSUM accumulation.
# Host: scatter-add each expert's [n_e, D] result into the [N, D] output.
#
# Shapes are hardcoded for B=4, S=2048, D=1024, H=4096, E=8 (spec), but the
# builder is parametric in the padded per-expert capacity C (known only after
# routing), so the Bass program is built after routing on every call.

import numpy as np
import ml_dtypes

NUM_EXPERTS = 8
TOP_K = 2
P = 128          # SBUF partitions
TB = 512         # token block (matmul moving free size)
NOUT = 512       # output free-dim tile (one PSUM bank of fp32)

_program_cache = {}


def _build_program(C, D, H):
    import concourse.mybir as mybir
    import concourse.tile as tile
    from concourse import bacc

    bf = mybir.dt.bfloat16
    f32 = mybir.dt.float32
    Gelu = mybir.ActivationFunctionType.Gelu_apprx_tanh
    Copy = mybir.ActivationFunctionType.Copy

    KD = D // P      # contraction chunks for mm1 (8)
    KH = H // P      # contraction chunks for mm2 (32)
    ND = D // NOUT   # output column tiles (2)

    nc = bacc.Bacc(None, target_bir_lowering=False, debug=False)
    xt = nc.declare_dram_parameter("xt", [D, C], bf, isOutput=False).ap()
    w1 = nc.declare_dram_parameter("w1", [D, H], bf, isOutput=False).ap()
    w2 = nc.declare_dram_parameter("w2", [H, D], bf, isOutput=False).ap()
    g = nc.declare_dram_parameter("g", [C, 1], f32, isOutput=False).ap()
    b1t = nc.declare_dram_parameter("b1t", [P, H // P], f32, isOutput=False).ap()
    y = nc.declare_dram_parameter("y", [C, D], f32, isOutput=True).ap()

    with tile.TileContext(nc) as tc:
        with (
            tc.tile_pool(name="weights", bufs=1) as wpool,
            tc.tile_pool(name="xin", bufs=2) as xpool,
            tc.tile_pool(name="hbuf", bufs=1) as hpool,
            tc.tile_pool(name="yout", bufs=3) as ypool,
            tc.tile_pool(name="gates", bufs=3) as gpool,
            tc.tile_pool(name="ph", bufs=4, space="PSUM") as php,
            tc.tile_pool(name="py", bufs=3, space="PSUM") as pyp,
        ):
            # resident weights: W1 as [P, KD, H] (lhsT chunks for mm1),
            # W2 as [P, KH, D] (rhs chunks for mm2)
            w1_sb = wpool.tile([P, KD, H], bf, tag="w1sb")
            w2_sb = wpool.tile([P, KH, D], bf, tag="w2sb")
            b1_sb = wpool.tile([P, H // P], f32, tag="b1sb")
            for k in range(KD):
                nc.sync.dma_start(w1_sb[:, k, :], w1[k * P:(k + 1) * P, :])
            for k in range(KH):
                nc.sync.dma_start(w2_sb[:, k, :], w2[k * P:(k + 1) * P, :])
            nc.sync.dma_start(b1_sb, b1t)

            nblocks = (C + TB - 1) // TB
            for b in range(nblocks):
                t0 = b * TB
                tbs = min(TB, C - t0)
                xt_blk = xpool.tile([P, KD, tbs], bf, tag="xt")
                for k in range(KD):
                    nc.sync.dma_start(
                        xt_blk[:, k, :], xt[k * P:(k + 1) * P, t0:t0 + tbs]
                    )
                # mm1: hT[m] = gelu(W1_chunk^T @ xT_block + b1)  -> [P, tbs] bf16
                hT = hpool.tile([P, KH, tbs], bf, tag="hT")
                for m in range(KH):
                    ph = php.tile([P, tbs], f32, tag="ph")
                    for k in range(KD):
                        nc.tensor.matmul(
                            ph,
                            w1_sb[:, k, m * P:(m + 1) * P],
                            xt_blk[:, k, :],
                            start=(k == 0),
                            stop=(k == KD - 1),
                        )
                    nc.scalar.activation(
                        hT[:, m, :], ph, Gelu, bias=b1_sb[:, m:m + 1]
                    )
                # mm2: y[tok_tile, n] = (hT_tok^T @ W2_chunk) * gate
                for mi in range(tbs // P):
                    tok = t0 + mi * P
                    gt = gpool.tile([P, 1], f32, tag="gt")
                    nc.sync.dma_start(gt, g[tok:tok + P, :])
                    for n in range(ND):
                        py = pyp.tile([P, NOUT], f32, tag="py")
                        for k in range(KH):
                            nc.tensor.matmul(
                                py,
                                hT[:, k, mi * P:(mi + 1) * P],
                                w2_sb[:, k, n * NOUT:(n + 1) * NOUT],
                                start=(k == 0),
                                stop=(k == KH - 1),
                            )
                        yt = ypool.tile([P, NOUT], f32, tag="yt")
                        nc.scalar.activation(yt, py, Copy, scale=gt)
                        nc.sync.dma_start(
                            y[tok:tok + P, n * NOUT:(n + 1) * NOUT], yt
                        )
    nc.compile()
    return nc


def kernel(x, Wr, W1, b1, W2, b2):
    from concourse.bass_utils import run_bass_kernel_spmd

    bf16 = ml_dtypes.bfloat16
    B, S, D = x.shape
    E, _, H = W1.shape
    N = B * S
    xm = np.ascontiguousarray(x.reshape(N, D), dtype=np.float32)

    # --- host router (mirrors reference fp32 arithmetic; softmax is
    # monotonic so top-k on probs == top-k on logits, ties broken by index)
    logits = xm @ Wr
    mx = logits.max(axis=1, keepdims=True)
    ex = np.exp(logits - mx)
    probs = ex / ex.sum(axis=1, keepdims=True)
    top_i = np.argsort(-probs, axis=1, kind="stable")[:, :TOP_K]

    idx = [np.where((top_i == e).any(axis=1))[0] for e in range(E)]
    counts = np.array([len(i) for i in idx])
    C = max(TB, int(-(-counts.max() // P) * P))  # pad to multiple of 128

    # --- dispatch: per-expert transposed token block [D, C] bf16
    xT = np.ascontiguousarray(xm.T).astype(bf16)  # [D, N]
    in_maps = []
    for e in range(E):
        xte = np.zeros((D, C), dtype=bf16)
        xte[:, :counts[e]] = xT[:, idx[e]]
        ge = np.zeros((C, 1), dtype=np.float32)
        ge[:counts[e], 0] = probs[idx[e], e]
        in_maps.append({
            "xt": xte,
            "w1": np.ascontiguousarray(W1[e], dtype=np.float32).astype(bf16),
            "w2": np.ascontiguousarray(W2[e], dtype=np.float32).astype(bf16),
            "g": ge,
            "b1t": np.ascontiguousarray(
                np.asarray(b1[e], dtype=np.float32).reshape(H // P, P).T
            ),
        })

    key = (C, D, H)
    if key not in _program_cache:
        _program_cache[key] = _build_program(C, D, H)
    nc = _program_cache[key]

    res = run_bass_kernel_spmd(nc, in_maps, core_ids=list(range(NUM_EXPERTS)))

    # --- combine: scatter-add gated expert outputs (indices unique per expert)
    out = np.zeros((N, D), dtype=np.float32)
    b2f = np.asarray(b2, dtype=np.float32)
    for e in range(E):
        ye = np.asarray(res.results[e]["y"][:counts[e]], dtype=np.float32)
        if b2f[e].any():
            ye = ye + probs[idx[e], e][:, None] * b2f[e]
        out[idx[e]] += ye
    return out.reshape(B, S, D)
